# revision 33
# baseline (speedup 1.0000x reference)
"""Causal self-attention (B=8, T=1024, C=1024, H=16) on 8 TRN2 NeuronCores.

Sharding: pure data-parallel over batch — core b computes batch element b
with fully replicated weights (B == n_cores, so no collectives needed).

Per-core dataflow (bf16 matmuls, fp32 accumulation):
  1. xT = transpose(x) via PE (fp32), evacuated to SBUF as bf16.
  2. qkvT[i, t] for i in [0, 2C): Q^T/K^T computed *transposed* so that
     per-head Q^T/K^T tiles [64, T] are directly the matmul operands for
     scores; bqkv folded into the PSUM evacuation (per-partition bias).
  3. V computed in natural [t, v] layout (lhsT = xT blocks); each head's
     64 columns stored with stride 66 plus a ones-column so the
     attention*V matmul also produces the softmax denominators for free.
  4. Scores computed transposed S^T[k, q] per (head, k-tile); causal
     structure skips k>q tiles; exp on ScalarE (scale=1/8 fused, no max
     subtraction needed — scores are O(1)); diagonal block masked
     multiplicatively after exp.
  5. Y^T[d, q] accumulated in PSUM over k-tiles (row 64 = sums), divided
     by sums via a DMA-broadcast reciprocal row, written to Y^T SBUF
     tiles that are directly the lhsT blocks of the output projection.
  6. out = Y @ Wproj + bproj, evacuated with a broadcast bias add.
"""

import numpy as np

import concourse.tile as tile
from concourse import bacc, mybir
from concourse.bass_utils import run_bass_kernel_spmd
from concourse.masks import make_identity

f32 = mybir.dt.float32
bf16 = mybir.dt.bfloat16
AF = mybir.ActivationFunctionType
ALU = mybir.AluOpType

B, T, C, H, HD = 8, 1024, 1024, 16, 64
P = 128
NT = T // P  # 8 token tiles
NS = C // P  # 8 contraction subtiles
W = 66  # per-head stride in V_sb: [64 vals][1 ones][1 pad]


def _build():
    nc = bacc.Bacc(trn_type="TRN2")
    x_d = nc.dram_tensor("x", (T, C), f32, kind="ExternalInput")
    wqkv_d = nc.dram_tensor("wqkv", (C, 3 * C), f32, kind="ExternalInput")
    bqkv_d = nc.dram_tensor("bqkv", (3 * C,), f32, kind="ExternalInput")
    wproj_d = nc.dram_tensor("wproj", (C, C), f32, kind="ExternalInput")
    bproj_d = nc.dram_tensor("bproj", (C,), f32, kind="ExternalInput")
    out_d = nc.dram_tensor("out", (T, C), f32, kind="ExternalOutput")

    with tile.TileContext(nc) as tc:
        with (
            tc.tile_pool(name="big", bufs=1) as big,
            tc.tile_pool(name="stage", bufs=2) as stage,
            tc.tile_pool(name="ptp", bufs=3) as ptp,
            tc.tile_pool(name="small", bufs=2) as small,
            tc.tile_pool(name="small1", bufs=1) as small1,
            tc.tile_pool(name="outp", bufs=2) as outp,
            tc.tile_pool(name="dramp", bufs=2, space="DRAM") as dramp,
            tc.tile_pool(name="pmm", bufs=2, space="PSUM") as pmm,
            tc.tile_pool(name="psp", bufs=2, space="PSUM") as psp,
            tc.tile_pool(name="pyp", bufs=1, space="PSUM") as pyp,
        ):
            # ---------------- constants ----------------
            ident = big.tile([P, P], f32, tag="ident")
            make_identity(nc, ident)
            # causal multiplicative mask for the transposed diagonal block:
            # cmask[k, q] = 1 if q >= k else 0
            cmask = big.tile([P, P], bf16, tag="cmask")
            nc.gpsimd.memset(cmask, 1.0)
            nc.gpsimd.affine_select(
                out=cmask,
                in_=cmask,
                compare_op=ALU.is_ge,
                fill=0.0,
                base=0,
                pattern=[[1, P]],
                channel_multiplier=-1,
            )
            # per-partition bias columns for the Q/K part of qkvT
            bqk_col = big.tile([P, 2 * C // P], f32, tag="bqk")
            nc.sync.dma_start(bqk_col, bqkv_d[: 2 * C].rearrange("(o p) -> p o", p=P))
            # broadcast bias rows (per free-dim column) for V and proj
            bias_v = big.tile([P, C], f32, tag="bias_v")
            nc.sync.dma_start(bias_v, bqkv_d[2 * C :][None, :].to_broadcast((P, C)))

            # ---------------- x first: DMA + PE transpose -> xT bf16 ----------------
            xt_sb = big.tile([P, NS, T], bf16, tag="xt")
            x_r = x_d[:, :].rearrange("(i p) c -> p i c", p=P)
            for i in range(NT):
                xst = stage.tile([P, C], f32, tag="stage", name=f"xst{i}")
                nc.sync.dma_start(xst, x_r[:, i, :])
                for jh in range(2):
                    tp = pmm.tile([P, 512], f32, tag="pmm")
                    for jj in range(4):
                        j = 4 * jh + jj
                        nc.tensor.transpose(
                            tp[:, jj * P : (jj + 1) * P],
                            xst[:, j * P : (j + 1) * P],
                            ident,
                        )
                    nc.vector.tensor_copy(
                        xt_sb[:, 4 * jh : 4 * jh + 4, i * P : (i + 1) * P],
                        tp.rearrange("p (j t) -> p j t", t=P),
                    )

            wqkv_sb = big.tile([P, NS, 3 * C], bf16, tag="wqkv")
            wq_r = wqkv_d[:, :].rearrange("(s p) i -> p s i", p=P)

            # ---------------- Q/K^T tiles (interleaved with attention) ----------
            qkt_sb = big.tile([P, 2 * C // P, T], bf16, tag="qkt")
            wproj_sb = big.tile([P, NS, C], bf16, tag="wproj")
            wp_r = wproj_d[:, :].rearrange("(s p) j -> p s j", p=P)

            def emit_qk(m):
                st = stage.tile([P, C], f32, tag="stage", name=f"wst{m}")
                st3 = st.rearrange("p (s c) -> p s c", c=P)
                nc.sync.dma_start(st3, wq_r[:, :, m * P : (m + 1) * P])
                nc.vector.tensor_copy(wqkv_sb[:, :, m * P : (m + 1) * P], st3)
                for ch in range(2):
                    ps = pmm.tile([P, 512], f32, tag="pmm", name=f"qk{m}_{ch}")
                    for s in range(NS):
                        nc.tensor.matmul(
                            ps,
                            wqkv_sb[:, s, m * P : (m + 1) * P],
                            xt_sb[:, s, ch * 512 : (ch + 1) * 512],
                            start=(s == 0),
                            stop=(s == NS - 1),
                        )
                    nc.vector.tensor_scalar_add(
                        qkt_sb[:, m, ch * 512 : (ch + 1) * 512],
                        ps,
                        bqk_col[:, m : m + 1],
                    )

            # pair 0's Q/K first so ScalarE's exp pipeline starts early
            emit_qk(0)
            emit_qk(C // P)

            # ---------------- load + cast V weight columns ----------------
            # (issued on the scalar-engine DMA ring so they don't delay the
            # x / Q/K loads on the sync ring)
            for s in range(NS):
                st = stage.tile([P, C], f32, tag="stage", name=f"vw{s}")
                nc.scalar.dma_start(st, wq_r[:, s, 2 * C : 3 * C])
                nc.vector.tensor_copy(wqkv_sb[:, s, 2 * C : 3 * C], st)

            # ---------------- V (natural layout, ones-augmented) ----------------
            v_sb = [big.tile([P, H * W], bf16, tag=f"v{i}", name=f"v{i}") for i in range(NT)]

            def emit_v(i):
                v3 = v_sb[i].rearrange("p (h w) -> p h w", w=W)
                nc.gpsimd.memset(v3[:, :, HD : HD + 1], 1.0)
                for ch in range(2):
                    ps = pmm.tile([P, 512], f32, tag="pmm", name=f"v{i}_{ch}")
                    for s in range(NS):
                        nc.tensor.matmul(
                            ps,
                            xt_sb[:, s, i * P : (i + 1) * P],
                            wqkv_sb[:, s, 2 * C + ch * 512 : 2 * C + (ch + 1) * 512],
                            start=(s == 0),
                            stop=(s == NS - 1),
                        )
                    nc.vector.tensor_tensor(
                        v3[:, 8 * ch : 8 * ch + 8, 0:HD],
                        ps.rearrange("p (h d) -> p h d", d=HD),
                        bias_v[:, ch * 512 : (ch + 1) * 512].rearrange(
                            "p (h d) -> p h d", d=HD
                        ),
                        ALU.add,
                    )

            # ---------------- attention ----------------
            # Heads are processed in pairs (even head at partitions 0..63,
            # odd at 64..127 of the same qkt tiles). Their score matmuls
            # contract over K=64 in different PE row-groups, so emitting
            # them adjacently lets the PE run both concurrently (row
            # tiling via base_partition). The odd head's AV is deferred to
            # a burst (its PT tiles persist in SBUF) so only one Y psum
            # tile is live at a time.
            yt_sb = [big.tile([P, T], bf16, tag=f"yt{g}", name=f"yt{g}") for g in range(NT)]
            proj_pre = {}

            def s_matmuls(sp, kt_h, qt_h, kt):
                q0 = kt * P
                if kt <= 3:
                    nc.tensor.matmul(
                        sp[:, q0:512], kt_h[:, q0 : q0 + P], qt_h[:, q0:512],
                        start=True, stop=True,
                    )
                    nc.tensor.matmul(
                        sp[:, 512:T], kt_h[:, q0 : q0 + P], qt_h[:, 512:T],
                        start=True, stop=True,
                    )
                else:
                    nc.tensor.matmul(
                        sp[:, q0:T], kt_h[:, q0 : q0 + P], qt_h[:, q0:T],
                        start=True, stop=True,
                    )

            def av_matmuls(ypA, ypB, pt_ap, h, kt, q_off):
                # ypA covers q columns [0,512), ypB [512,T); pt_ap covers
                # q columns [q_off, T); accumulate over kt
                q0 = kt * P
                lhsT_v = v_sb[kt][:, h * W : h * W + HD + 1]  # [128, 65]
                if kt <= 3:
                    nc.tensor.matmul(
                        ypA[0 : HD + 1, q0:512], lhsT_v, pt_ap[:, q0 - q_off : 512 - q_off],
                        start=(kt == 0), stop=(kt == 3),
                    )
                    nc.tensor.matmul(
                        ypB[0 : HD + 1, 0:512], lhsT_v, pt_ap[:, 512 - q_off : T - q_off],
                        start=(kt == 0), stop=(kt == NT - 1),
                    )
                else:
                    nc.tensor.matmul(
                        ypB[0 : HD + 1, q0 - 512 : 512], lhsT_v, pt_ap[:, q0 - q_off : T - q_off],
                        start=False, stop=(kt == NT - 1),
                    )

            def finish_head(ypA, ypB, h, fast=False):
                # evacuate unnormalized Y + sums immediately to free the
                # Y psum tiles; the reciprocal chain then runs off the
                # PE/ACT critical path entirely in SBUF/DRAM.
                sums = small1.tile([HD + 1, T], f32, tag="sums", name=f"sums{h}")
                nc.vector.tensor_copy(sums[HD : HD + 1, 0:512], ypA[HD : HD + 1, 0:512])
                nc.vector.tensor_copy(sums[HD : HD + 1, 512:T], ypB[HD : HD + 1, 0:512])
                yu = small.tile([HD, T], bf16, tag="yu", name=f"yu{h}")
                nc.vector.tensor_copy(yu[:, 0:512], ypA[0:HD, 0:512])
                nc.vector.tensor_copy(yu[:, 512:T], ypB[0:HD, 0:512])
                # reshape sums to [64, 16] via DRAM so the reciprocal runs
                # on 64 lanes instead of 1, then broadcast recips to [64,T].
                dma = nc.sync.dma_start if fast else nc.gpsimd.dma_start
                if True:
                    scr = dramp.tile([T], f32, tag="scr", name=f"scr{h}")
                    dma(scr[None, :], sums[HD : HD + 1, :])
                    s64 = small1.tile([HD, T // HD], f32, tag="s64", name=f"s64_{h}")
                    dma(s64, scr.rearrange("(p e) -> p e", p=HD))
                    r64 = small1.tile([HD, T // HD], bf16, tag="r64", name=f"r64_{h}")
                    with nc.allow_low_precision("softmax recips in bf16 (tol 2e-2)"):
                        nc.vector.reciprocal(r64, s64)
                    scr2 = dramp.tile([T], bf16, tag="scr2", name=f"scr2_{h}")
                    dma(scr2.rearrange("(p e) -> p e", p=HD), r64)
                    r_sb = small.tile([HD, T], bf16, tag="r", name=f"r{h}")
                    dma(r_sb, scr2[None, :].to_broadcast((HD, T)))
                r_use = r_sb
                g = h // 2
                tt = nc.vector.tensor_tensor
                if h % 2 == 0:
                    tt(yt_sb[g][0:HD, :], yu, r_use, ALU.mult)
                else:
                    ytmp = small1.tile([HD, T], bf16, tag="ytmp", name=f"ytmp{h}")
                    tt(ytmp, yu, r_use, ALU.mult)
                    # partition shift 0..63 -> 64..127 via SBUF-to-SBUF DMA
                    dma(yt_sb[g][HD:P, :], ytmp)

            for g in range(NT):
                h0, h1 = 2 * g, 2 * g + 1
                m = g
                if g > 0:
                    emit_qk(m)
                    emit_qk((C // P) + m)
                if g == 2:
                    # wproj load emitted early enough to overlap attention
                    for s in range(NS):
                        st = stage.tile([P, C], f32, tag="stage", name=f"wpst{s}")
                        nc.sync.dma_start(st, wp_r[:, s, :])
                        nc.vector.tensor_copy(wproj_sb[:, s, :], st)
                qt0 = qkt_sb[0:HD, m, :]
                kt0 = qkt_sb[0:HD, (C // P) + m, :]
                qt1 = qkt_sb[HD:P, m, :]
                kt1 = qkt_sb[HD:P, (C // P) + m, :]
                yp = pyp.tile([P, T], f32, tag="py", name=f"yp{h0}")
                ypA, ypB = yp[:, 0:512], yp[:, 512:T]
                pt1s = []
                for kt in range(NT):
                    if g == 0:
                        emit_v(kt)
                    q0 = kt * P
                    sp0 = psp.tile([P, T], f32, tag="ps", name=f"sp0_{g}_{kt}")
                    sp1 = psp.tile([P, T], f32, tag="ps", name=f"sp1_{g}_{kt}")
                    s_matmuls(sp0, kt0, qt0, kt)
                    s_matmuls(sp1, kt1, qt1, kt)
                    pt0 = ptp.tile([P, T], bf16, tag="pt", name=f"pt0_{g}_{kt}")
                    nc.scalar.activation(pt0[:, q0:T], sp0[:, q0:T], AF.Exp, scale=0.125)
                    pt1 = small1.tile([P, T - q0], bf16, tag=f"pt1_{kt}", name=f"pt1_{g}_{kt}")
                    nc.scalar.activation(pt1, sp1[:, q0:T], AF.Exp, scale=0.125)
                    # mask the diagonal block (k > q within the block -> 0)
                    nc.vector.tensor_tensor(
                        pt0[:, q0 : q0 + P], pt0[:, q0 : q0 + P], cmask, ALU.mult
                    )
                    nc.vector.tensor_tensor(
                        pt1[:, 0:P], pt1[:, 0:P], cmask, ALU.mult
                    )
                    av_matmuls(ypA, ypB, pt0, h0, kt, 0)
                    pt1s.append(pt1)
                finish_head(ypA, ypB, h0, fast=(g >= NT - 3))
                yp1 = pyp.tile([P, T], f32, tag="py", name=f"yp{h1}")
                yp1A, yp1B = yp1[:, 0:512], yp1[:, 512:T]
                for kt in range(NT):
                    av_matmuls(yp1A, yp1B, pt1s[kt], h1, kt, kt * P)
                if g == NT - 1:
                    # pre-accumulate the first two proj groups over heads
                    # 0..13 while the last pair's normalize chain drains --
                    # fills the PE stall window and keeps the clock warm
                    for ch in range(2):
                        ps = pmm.tile([P, 512], f32, tag="pmm", name=f"projpre{ch}")
                        for gg in range(NT - 1):
                            nc.tensor.matmul(
                                ps,
                                yt_sb[gg][:, 0:P] if False else yt_sb[gg][:, 0:P],
                                wproj_sb[:, gg, ch * 512 : (ch + 1) * 512],
                                start=(gg == 0),
                                stop=False,
                            )
                        proj_pre[ch] = ps
                finish_head(yp1A, yp1B, h1, fast=(g >= NT - 3))

            # ---------------- output projection ----------------
            # reuse the V bias tile for the proj bias (V phase is done)
            bias_o = bias_v
            nc.sync.dma_start(bias_o, bproj_d[:][None, :].to_broadcast((P, C)))
            out_r = out_d[:, :].rearrange("(i p) j -> p i j", p=P)
            for i in range(NT):
                for ch in range(2):
                    if i == 0:
                        ps = proj_pre[ch]
                        nc.tensor.matmul(
                            ps,
                            yt_sb[NT - 1][:, 0:P],
                            wproj_sb[:, NT - 1, ch * 512 : (ch + 1) * 512],
                            start=False,
                            stop=True,
                        )
                    else:
                        ps = pmm.tile([P, 512], f32, tag="pmm", name=f"proj{i}_{ch}")
                        for g in range(NT):
                            nc.tensor.matmul(
                                ps,
                                yt_sb[g][:, i * P : (i + 1) * P],
                                wproj_sb[:, g, ch * 512 : (ch + 1) * 512],
                                start=(g == 0),
                                stop=(g == NT - 1),
                            )
                    ot = outp.tile([P, 512], f32, tag="out")
                    nc.vector.tensor_tensor(
                        ot, ps, bias_o[:, ch * 512 : (ch + 1) * 512], ALU.add
                    )
                    nc.sync.dma_start(out_r[:, i, ch * 512 : (ch + 1) * 512], ot)

    nc.compile()
    return nc


_NC = None


def _get_nc():
    global _NC
    if _NC is None:
        _NC = _build()
    return _NC


def _in_maps(x, Wqkv, bqkv, Wproj, bproj):
    x = np.ascontiguousarray(np.asarray(x, dtype=np.float32))
    shared = {
        "wqkv": np.ascontiguousarray(np.asarray(Wqkv, dtype=np.float32)),
        "bqkv": np.ascontiguousarray(np.asarray(bqkv, dtype=np.float32)),
        "wproj": np.ascontiguousarray(np.asarray(Wproj, dtype=np.float32)),
        "bproj": np.ascontiguousarray(np.asarray(bproj, dtype=np.float32)),
    }
    return [{"x": np.ascontiguousarray(x[b]), **shared} for b in range(B)]


def run(x, Wqkv, bqkv, Wproj, bproj, **run_kwargs):
    """Run on 8 cores; returns (output [B,T,C] fp32, BassKernelResults)."""
    nc = _get_nc()
    res = run_bass_kernel_spmd(
        nc, _in_maps(x, Wqkv, bqkv, Wproj, bproj), core_ids=list(range(B)), **run_kwargs
    )
    out = np.stack([res.results[b]["out"] for b in range(B)]).astype(np.float32)
    return out, res


def kernel(x, Wqkv, bqkv, Wproj, bproj, n_head=None, **_ignored):
    out, _ = run(x, Wqkv, bqkv, Wproj, bproj)
    return out
